# revision 11
# baseline (speedup 1.0000x reference)
"""GQA attention block (B=2, S=2048, DIM=4096, 32 Q heads / 8 KV heads, HD=128,
RoPE + causal softmax + output projection) on 8 trn2 NeuronCores.

Sharding: 8 cores = 2 batches x 4 head-groups. Core c handles batch c%2 and
head-group c//2 (8 Q heads, 2 KV heads). Each core computes a full-size
[S, DIM] partial of the output projection (its heads' contribution); the host
sums the 4 group-partials per batch.

v2 (bf16): all matmul operands are bf16 (PSUM accumulation stays fp32), which
halves DMA traffic and SBUF footprint vs fp32r at the same PE rate:
  - wq is fully resident (8MB bf16), so the Q projection is a single pass over
    x with full 32-chunk PSUM accumulation (no 2-level accumulation, no 4x
    re-streaming of x).
  - softmax denominators: exp tiles are accumulated on the vector engine into
    a per-(head, q-chunk) fp32 accumulator; ONE ones-column matmul per block
    replaces the per-key-tile ones matmuls (320 -> 32 PE passes).
  - weight/table DMAs are chunked and interleaved with the first compute loop
    so the PE starts within a few microseconds.
  - wq prefetches during phase A; wo prefetches during phase S.
Layout is transposed end-to-end as in v1 (head_dim on partitions; scores
computed as S^T[k, q]; PV output attn^T[d, q] feeds WO directly).
"""

import math
import os
import sys
from contextlib import ExitStack
from dataclasses import dataclass

import numpy as np

sys.path.insert(0, "/opt/trn_rl_repo")

import concourse.bass as bass  # noqa: E402
import concourse.mybir as mybir  # noqa: E402
import concourse.tile as tile  # noqa: E402
from concourse import bacc  # noqa: E402

F32 = mybir.dt.float32
F32R = mybir.dt.float32r
BF16 = mybir.dt.bfloat16
P = 128


@dataclass(frozen=True)
class Cfg:
    S: int = 2048      # sequence length
    DIM: int = 4096    # model dim (contraction for projections)
    NH_L: int = 8      # q heads per core
    NKV_L: int = 2     # kv heads per core
    HD: int = 128      # head dim (must be P)
    TQ: int = 512      # token/query chunk (PSUM free dim)

    @property
    def CCH(self):  # contraction chunks
        return self.DIM // P

    @property
    def NT(self):  # token chunks
        return self.S // self.TQ

    @property
    def NKT(self):  # key tiles
        return self.S // P

    @property
    def RT(self):  # key tiles per token chunk
        return self.TQ // P

    @property
    def NREP(self):
        return self.NH_L // self.NKV_L


def build_program(cfg: Cfg, debug: bool = False) -> bass.Bass:
    nc = bacc.Bacc("TRN2", target_bir_lowering=False)
    S, DIM, NH_L, NKV_L, HD, TQ = cfg.S, cfg.DIM, cfg.NH_L, cfg.NKV_L, cfg.HD, cfg.TQ
    CCH, NT, RT = cfg.CCH, cfg.NT, cfg.RT
    MULT = mybir.AluOpType.mult

    if debug:
        dbg_kt = nc.dram_tensor("dbg_kt", [P, NKV_L, S], BF16, kind="ExternalOutput")
        dbg_v = nc.dram_tensor("dbg_v", [P, cfg.NKT, NKV_L * HD], BF16,
                               kind="ExternalOutput")
        dbg_qt = nc.dram_tensor("dbg_qt", [P, NH_L, S], BF16, kind="ExternalOutput")
        dbg_at = nc.dram_tensor("dbg_at", [P, NH_L, S], BF16, kind="ExternalOutput")

    xT_d = nc.dram_tensor("xT", [DIM, S], BF16, kind="ExternalInput")
    wq_d = nc.dram_tensor("wq", [DIM, NH_L * HD], BF16, kind="ExternalInput")
    wk_d = nc.dram_tensor("wk", [DIM, NKV_L * HD], BF16, kind="ExternalInput")
    wv_d = nc.dram_tensor("wv", [DIM, NKV_L * HD], BF16, kind="ExternalInput")
    wo_d = nc.dram_tensor("wo", [NH_L * HD, DIM], BF16, kind="ExternalInput")
    cosq_d = nc.dram_tensor("cosq", [P, S], BF16, kind="ExternalInput")
    sinq_d = nc.dram_tensor("sinq", [P, S], BF16, kind="ExternalInput")
    cosk_d = nc.dram_tensor("cosk", [P, S], BF16, kind="ExternalInput")
    sink_d = nc.dram_tensor("sink", [P, S], BF16, kind="ExternalInput")
    maskT_d = nc.dram_tensor("maskT", [P, P], F32, kind="ExternalInput")
    out_d = nc.dram_tensor("out", [S, DIM], F32, kind="ExternalOutput")

    xT_r = xT_d.ap().rearrange("(co ci) t -> ci co t", ci=P)
    wq_r = wq_d.ap().rearrange("(co ci) d -> ci co d", ci=P)
    wk_r = wk_d.ap().rearrange("(co ci) d -> ci co d", ci=P)
    wv_r = wv_d.ap().rearrange("(co ci) d -> ci co d", ci=P)
    wo_r = wo_d.ap().rearrange("(dc p) m -> p dc m", p=P)

    def mm(out, lhsT, rhs, start, stop):
        nc.tensor.matmul(out, lhsT, rhs, start=start, stop=stop)

    with tile.TileContext(nc) as tc, ExitStack() as top:
        const = top.enter_context(tc.tile_pool(name="const", bufs=1))
        maskT_sb = const.tile([P, P], F32)
        nc.sync.dma_start(maskT_sb[:], maskT_d.ap())
        ones_col = const.tile([P, 1], BF16)
        nc.vector.memset(ones_col[:], 1.0)

        kvp = top.enter_context(tc.tile_pool(name="kvp", bufs=1))
        KT_sb = kvp.tile([P, NKV_L, S], BF16)
        V_sb = kvp.tile([P, cfg.NKT, NKV_L * HD], BF16)

        # persistent across A+Q (alloc order: qtp below wqp so wqp can free
        # first; left-side stack must stay LIFO)
        qtp = tc.alloc_tile_pool(name="qtp", bufs=1)
        qt_sb = qtp.tile([P, NH_L, S], BF16)
        wqp = tc.alloc_tile_pool(name="wqp", bufs=1)
        wq_sb = wqp.tile([P, CCH, NH_L * HD], BF16)
        cosq_sb = wqp.tile([P, S], BF16)
        sinq_sb = wqp.tile([P, S], BF16)

        def rope_inplace(dst, cos_sl, sin_sl, tmp_pool):
            # dst [P, n] bf16 in SBUF: dst = dst*cos + swap_halves(dst)*sin
            n = dst.shape[-1]
            tmp = tmp_pool.tile([P, TQ], BF16, tag="ropetmp", name="ropetmp")
            t = tmp[:, :n]
            nc.sync.dma_start(t[0:64], dst[64:128])
            nc.sync.dma_start(t[64:128], dst[0:64])
            nc.vector.tensor_tensor(t, t, sin_sl, MULT)
            nc.vector.tensor_tensor(dst, dst, cos_sl, MULT)
            nc.vector.tensor_add(dst, dst, t)

        # ---------------- Phase A: K^T and V projections (+ RoPE on K) -----
        # wk/wv chunk DMAs are interleaved into tn=0's c-loop (just ahead of
        # use); cos/sin tables load per-tn; wq + q-tables prefetch during
        # tn>=1 so phase Q starts immediately.
        with ExitStack() as ctx:
            wkvp = ctx.enter_context(tc.tile_pool(name="wkvp", bufs=1))
            ktab = ctx.enter_context(tc.tile_pool(name="ktab", bufs=1))
            xap = ctx.enter_context(tc.tile_pool(name="xap", bufs=8))
            rtp = ctx.enter_context(tc.tile_pool(name="rtp", bufs=2))
            pka = ctx.enter_context(tc.tile_pool(name="pka", bufs=2, space="PSUM"))
            pva = ctx.enter_context(tc.tile_pool(name="pva", bufs=1, space="PSUM"))

            wk_sb = wkvp.tile([P, CCH, NKV_L * HD], BF16)
            wv_sb = wkvp.tile([P, CCH, NKV_L * HD], BF16)
            cosk_sb = ktab.tile([P, S], BF16)
            sink_sb = ktab.tile([P, S], BF16)

            # wq prefetch schedule: (tn, c) -> slice of chunks to fetch
            LOOKAHEAD = 2
            for tn in range(NT):
                tsl = slice(tn * TQ, (tn + 1) * TQ)
                # per-tn table loads (only the slice this tn ropes)
                nc.sync.dma_start(cosk_sb[:, tsl], cosk_d.ap()[:, tsl])
                nc.sync.dma_start(sink_sb[:, tsl], sink_d.ap()[:, tsl])
                psk = [pka.tile([P, TQ], F32, tag=f"psk{d}", name=f"psk{d}")
                       for d in range(NKV_L)]
                psv = [pva.tile([P, NKV_L * HD], F32, tag=f"psv{j}", name=f"psv{j}")
                       for j in range(RT)]
                for c in range(CCH):
                    if tn == 0:
                        # JIT weight chunks, LOOKAHEAD ahead of use
                        if c == 0:
                            for cc in range(LOOKAHEAD + 1):
                                nc.sync.dma_start(wk_sb[:, cc, :], wk_r[:, cc, :])
                                nc.sync.dma_start(wv_sb[:, cc, :], wv_r[:, cc, :])
                        elif c + LOOKAHEAD < CCH:
                            cc = c + LOOKAHEAD
                            nc.sync.dma_start(wk_sb[:, cc, :], wk_r[:, cc, :])
                            nc.sync.dma_start(wv_sb[:, cc, :], wv_r[:, cc, :])
                    else:
                        # spread wq + q-table prefetch over tn>=1 iterations
                        step = (tn - 1) * CCH + c  # 0..95
                        if step < CCH:
                            nc.sync.dma_start(wq_sb[:, step, :], wq_r[:, step, :])
                        elif step < CCH + NT:
                            t2 = step - CCH
                            s2 = slice(t2 * TQ, (t2 + 1) * TQ)
                            nc.sync.dma_start(cosq_sb[:, s2], cosq_d.ap()[:, s2])
                        elif step < CCH + 2 * NT:
                            t2 = step - CCH - NT
                            s2 = slice(t2 * TQ, (t2 + 1) * TQ)
                            nc.sync.dma_start(sinq_sb[:, s2], sinq_d.ap()[:, s2])
                    xt = xap.tile([P, TQ], BF16, tag="xa", name="xa")
                    nc.sync.dma_start(xt[:], xT_r[:, c, tsl])
                    st, sp = c == 0, c == CCH - 1
                    for d in range(NKV_L):
                        mm(psk[d][:], wk_sb[:, c, d * HD:(d + 1) * HD], xt[:], st, sp)
                    for j in range(RT):
                        mm(psv[j][:], xt[:, j * P:(j + 1) * P], wv_sb[:, c, :], st, sp)
                for j in range(RT):
                    nc.scalar.copy(V_sb[:, tn * RT + j, :], psv[j][:])
                for d in range(NKV_L):
                    nc.scalar.copy(KT_sb[:, d, tsl], psk[d][:])
                    rope_inplace(KT_sb[:, d, tsl], cosk_sb[:, tsl], sink_sb[:, tsl], rtp)

        # ---------------- Phase Q: Q^T projection (+ RoPE on Q) ------------
        # Single pass: full 32-chunk PSUM accumulation. Heads processed in two
        # halves of 4 (4 PSUM banks each, double-buffered = 8 banks); x tiles
        # for the current tn stay resident (32 chunk tiles) and are read by
        # both halves.
        with ExitStack() as ctx:
            xqp = ctx.enter_context(tc.tile_pool(name="xqp", bufs=1))
            rtq = ctx.enter_context(tc.tile_pool(name="rtq", bufs=2))
            pqa = ctx.enter_context(tc.tile_pool(name="pqa", bufs=2, space="PSUM"))

            xts = {}
            for tn in range(NT):
                tsl = slice(tn * TQ, (tn + 1) * TQ)
                for c in range(CCH):
                    xt = xqp.tile([P, TQ], BF16, tag=f"xq{c}", name=f"xq{c}")
                    nc.sync.dma_start(xt[:], xT_r[:, c, tsl])
                    xts[c] = xt
                for hh in range(2):
                    psq = [pqa.tile([P, TQ], F32, tag=f"psq{h}", name=f"psq{h}")
                           for h in range(4)]
                    for c in range(CCH):
                        st, sp = c == 0, c == CCH - 1
                        for h in range(4):
                            hq = hh * 4 + h
                            mm(psq[h][:], wq_sb[:, c, hq * HD:(hq + 1) * HD],
                               xts[c][:], st, sp)
                    for h in range(4):
                        hq = hh * 4 + h
                        nc.scalar.copy(qt_sb[:, hq, tsl], psq[h][:])
                        rope_inplace(qt_sb[:, hq, tsl], cosq_sb[:, tsl],
                                     sinq_sb[:, tsl], rtq)

        wqp.release()

        if debug:
            nc.sync.dma_start(dbg_kt.ap(), KT_sb[:])
            nc.sync.dma_start(dbg_v.ap(), V_sb[:])
            nc.sync.dma_start(dbg_qt.ap(), qt_sb[:])

        # ---------------- Phase S: attention per head ----------------------
        # Per (q-chunk, head): scores^T -> exp (scalar) -> PV accumulation;
        # exp tiles also accumulate on the vector engine into acc[128, TQ];
        # one ones-column matmul per block turns acc into the denominators.
        # wo prefetches in the background.
        atp = tc.alloc_tile_pool(name="atp", bufs=1, side="right")
        attnT_sb = atp.tile([P, NH_L, S], BF16)
        wop = tc.alloc_tile_pool(name="wop", bufs=1, side="right")
        wo_sb = wop.tile([P, NH_L, DIM], BF16)
        with ExitStack() as ctx:
            ptp = ctx.enter_context(tc.tile_pool(name="ptp", bufs=6))
            acp = ctx.enter_context(tc.tile_pool(name="acp", bufs=3))
            bcp = ctx.enter_context(tc.tile_pool(name="bcp", bufs=2))
            psc = ctx.enter_context(tc.tile_pool(name="psc", bufs=3, space="PSUM"))
            pso = ctx.enter_context(tc.tile_pool(name="pso", bufs=2, space="PSUM"))
            pss = ctx.enter_context(tc.tile_pool(name="pss", bufs=2, space="PSUM"))

            def epilogue(pend):
                # denominators for a finished block: ones-matmul on the
                # vector-accumulated exp tile, reciprocal, broadcast,
                # normalize into attnT. Deferred one block so the PE never
                # waits on the vector accumulate chain.
                h, qsl, ps_out, acc = pend
                ps_sum = pss.tile([1, TQ], F32, tag="pssum", name="pssum")
                mm(ps_sum[:], ones_col[:], acc[:], True, True)
                rrow = bcp.tile([1, TQ], F32, tag="rrow", name="rrow")
                nc.vector.reciprocal_approx_fast(out=rrow[:], in_=ps_sum[:])
                bc_sb = bcp.tile([P, TQ], F32, tag="bcsb", name="bcsb")
                nc.gpsimd.partition_broadcast(bc_sb[:], rrow[:])
                nc.vector.tensor_tensor(attnT_sb[:, h, qsl], ps_out[:],
                                        bc_sb[:], MULT)

            pending = None
            blk = 0
            for qc in range(NT):
                for h in range(NH_L):
                    # background wo prefetch (two [P, 512] slabs per block;
                    # 64 slabs total = 8 heads x 8 column chunks)
                    for s in (2 * blk, 2 * blk + 1):
                        if s < NH_L * (DIM // TQ):
                            dc, mc = s % NH_L, s // NH_L
                            msl = slice(mc * TQ, (mc + 1) * TQ)
                            nc.sync.dma_start(wo_sb[:, dc, msl], wo_r[:, dc, msl])
                    blk += 1
                    g = h // cfg.NREP
                    qsl = slice(qc * TQ, (qc + 1) * TQ)
                    ps_out = pso.tile([P, TQ], F32, tag="psout", name="psout")
                    acc = acp.tile([P, TQ], BF16, tag="acc", name="acc")
                    nkt = (qc + 1) * RT
                    for kt in range(nkt):
                        ps_sc = psc.tile([P, TQ], F32, tag="pssc", name="pssc")
                        mm(ps_sc[:], KT_sb[:, g, kt * P:(kt + 1) * P],
                           qt_sb[:, h, qsl], True, True)
                        if kt >= qc * RT:
                            qoff = (kt - qc * RT) * P
                            if qoff > 0:
                                nc.vector.memset(ps_sc[:, 0:qoff], -1e9)
                            nc.vector.tensor_add(ps_sc[:, qoff:qoff + P],
                                                 ps_sc[:, qoff:qoff + P],
                                                 maskT_sb[:])
                        pt = ptp.tile([P, TQ], BF16, tag="pt", name="pt")
                        nc.scalar.activation(pt[:], ps_sc[:],
                                             mybir.ActivationFunctionType.Exp)
                        st, sp = kt == 0, kt == nkt - 1
                        mm(ps_out[:], V_sb[:, kt, g * HD:(g + 1) * HD], pt[:], st, sp)
                        if kt == 0:
                            nc.vector.tensor_copy(acc[:], pt[:])
                        else:
                            nc.vector.tensor_add(acc[:], acc[:], pt[:])
                        if kt == min(1, nkt - 1) and pending is not None:
                            epilogue(pending)
                            pending = None
                    pending = (h, qsl, ps_out, acc)
            epilogue(pending)

        if debug:
            nc.sync.dma_start(dbg_at.ap(), attnT_sb[:])

        qtp.release()

        # ---------------- Phase W: output projection -----------------------
        with ExitStack() as ctx:
            owp = ctx.enter_context(tc.tile_pool(name="owp", bufs=3, side="right"))
            psw = ctx.enter_context(tc.tile_pool(name="psw", bufs=4, space="PSUM"))

            for mc in range(DIM // TQ):
                msl = slice(mc * TQ, (mc + 1) * TQ)
                for tb in range(S // P):
                    ps_w = psw.tile([P, TQ], F32, tag="psw", name="psw")
                    for dc in range(NH_L):
                        mm(ps_w[:], attnT_sb[:, dc, tb * P:(tb + 1) * P],
                           wo_sb[:, dc, msl], dc == 0, dc == NH_L - 1)
                    ot = owp.tile([P, TQ], F32, tag="ot", name="ot")
                    nc.scalar.copy(ot[:], ps_w[:])
                    nc.sync.dma_start(out_d.ap()[tb * P:(tb + 1) * P, msl], ot[:])

        wop.release()
        atp.release()

    nc.compile()
    return nc


# ---------------------------------------------------------------------------
# Host side
# ---------------------------------------------------------------------------

_HALF_PERM = np.concatenate([np.arange(0, P, 2), np.arange(1, P, 2)])

LAST_EXEC_NS = None
LAST_RESULTS = None


def _host_prep(cfg: Cfg, x, wq, wk, wv, wo, freqs_cos, freqs_sin):
    """Build the 8 per-core input maps. Core c: batch c % 2, group c // 2."""
    import ml_dtypes
    BF = ml_dtypes.bfloat16

    B = x.shape[0]
    n_groups = wq.shape[1] // (cfg.NH_L * cfg.HD)
    hd = cfg.HD

    cosT = np.ascontiguousarray(freqs_cos.T.astype(np.float32))  # [HD/2, S]
    sinT = np.ascontiguousarray(freqs_sin.T.astype(np.float32))
    sc = np.float32(1.0 / math.sqrt(hd))
    cosq = (np.concatenate([cosT, cosT], 0) * sc).astype(BF)
    sinq = (np.concatenate([-sinT, sinT], 0) * sc).astype(BF)
    cosk = np.concatenate([cosT, cosT], 0).astype(BF)
    sink = np.concatenate([-sinT, sinT], 0).astype(BF)
    maskT = np.tril(np.full((P, P), -1e9, np.float32), -1)

    xT = [np.ascontiguousarray(x[b].T).astype(BF) for b in range(B)]

    def permute_cols(w, nheads):
        w = w.reshape(cfg.DIM, nheads, hd)[:, :, _HALF_PERM]
        return np.ascontiguousarray(w.reshape(cfg.DIM, nheads * hd)).astype(BF)

    in_maps = []
    qcols = cfg.NH_L * hd
    kcols = cfg.NKV_L * hd
    for c in range(B * n_groups):
        b, g = c % B, c // B
        in_maps.append(dict(
            xT=xT[b],
            wq=permute_cols(wq[:, g * qcols:(g + 1) * qcols], cfg.NH_L),
            wk=permute_cols(wk[:, g * kcols:(g + 1) * kcols], cfg.NKV_L),
            wv=np.ascontiguousarray(wv[:, g * kcols:(g + 1) * kcols]).astype(BF),
            wo=np.ascontiguousarray(wo[g * qcols:(g + 1) * qcols, :]).astype(BF),
            cosq=cosq, sinq=sinq, cosk=cosk, sink=sink, maskT=maskT,
        ))
    return in_maps


def kernel(x, wq, wk, wv, wo, freqs_cos, freqs_sin, mask, start_pos=0):
    global LAST_EXEC_NS, LAST_RESULTS
    x = np.asarray(x, np.float32)
    wq = np.asarray(wq, np.float32)
    wk = np.asarray(wk, np.float32)
    wv = np.asarray(wv, np.float32)
    wo = np.asarray(wo, np.float32)
    freqs_cos = np.asarray(freqs_cos, np.float32)
    freqs_sin = np.asarray(freqs_sin, np.float32)

    cfg = Cfg()
    B = x.shape[0]
    n_groups = 4
    in_maps = _host_prep(cfg, x, wq, wk, wv, wo, freqs_cos, freqs_sin)

    from concourse.bass_utils import run_bass_kernel_spmd

    debug = bool(int(os.environ.get("KERNEL_DEBUG", "0")))
    nc = build_program(cfg, debug=debug)
    trace = bool(int(os.environ.get("KERNEL_TRACE", "0")))
    res = run_bass_kernel_spmd(nc, in_maps, core_ids=list(range(len(in_maps))),
                               trace=trace)
    LAST_EXEC_NS = res.exec_time_ns
    LAST_RESULTS = res

    out = np.zeros((B, cfg.S, cfg.DIM), np.float32)
    for c in range(B * n_groups):
        b = c % B
        out[b] += res.results[c]["out"]
    return out


# revision 20
# speedup vs baseline: 1.1855x; 1.1855x over previous
"""GQA attention block (B=2, S=2048, DIM=4096, 32 Q heads / 8 KV heads, HD=128,
RoPE + causal softmax + output projection) on 8 trn2 NeuronCores.

Sharding: 8 cores = 2 batches x 4 head-groups. Core c handles batch c%2 and
head-group c//2 (8 Q heads, 2 KV heads). Each core computes a full-size
[S, DIM] partial of the output projection (its heads' contribution); the host
sums the 4 group-partials per batch.

v3 (hybrid): fp32r matmuls for the projections (empirically ~233ns vs bf16's
~266ns per 512-free matmul); bf16 only on the attention inner path where it
buys vector throughput:
  - phase S is software-pipelined: PV(kt) is emitted L=3 score-matmuls after
    scores(kt), so the PE never waits on the scalar engine's exp latency.
  - causal masks are applied multiplicatively to the exp tile (bf16, 2x DVE)
    instead of additively to the PSUM scores, off the exp critical path.
  - softmax denominators: exp tiles accumulate on the vector engine (bf16, 2x)
    into acc; an all-ones [128,128] stationary matmul turns acc into
    partition-broadcast denominators in PSUM (no gpsimd in the chain); the
    whole epilogue is deferred one block.
  - all weight streams are JIT-chunked (wk/wv inside tn=0's c-loop, wq pieces
    inside the group loop, wo slabs inside phase W) so the PE starts within a
    few us and phase seams are ~0.
"""

import math
import os
import sys
from contextlib import ExitStack
from dataclasses import dataclass

import numpy as np

sys.path.insert(0, "/opt/trn_rl_repo")

import concourse.bass as bass  # noqa: E402
import concourse.mybir as mybir  # noqa: E402
import concourse.tile as tile  # noqa: E402
from concourse import bacc  # noqa: E402

F32 = mybir.dt.float32
F32R = mybir.dt.float32r
BF16 = mybir.dt.bfloat16
P = 128


@dataclass(frozen=True)
class Cfg:
    S: int = 2048      # sequence length
    DIM: int = 4096    # model dim (contraction for projections)
    NH_L: int = 8      # q heads per core
    NKV_L: int = 2     # kv heads per core
    HD: int = 128      # head dim (must be P)
    TQ: int = 512      # token/query chunk (PSUM free dim)

    @property
    def CCH(self):  # contraction chunks
        return self.DIM // P

    @property
    def NT(self):  # token chunks
        return self.S // self.TQ

    @property
    def NKT(self):  # key tiles
        return self.S // P

    @property
    def RT(self):  # key tiles per token chunk
        return self.TQ // P

    @property
    def NREP(self):
        return self.NH_L // self.NKV_L


def build_program(cfg: Cfg, debug: bool = False) -> bass.Bass:
    nc = bacc.Bacc("TRN2", target_bir_lowering=False)
    S, DIM, NH_L, NKV_L, HD, TQ = cfg.S, cfg.DIM, cfg.NH_L, cfg.NKV_L, cfg.HD, cfg.TQ
    CCH, NT, RT = cfg.CCH, cfg.NT, cfg.RT
    MULT = mybir.AluOpType.mult

    xT_d = nc.dram_tensor("xT", [DIM, S], F32R, kind="ExternalInput")
    wq_d = nc.dram_tensor("wq", [DIM, NH_L * HD], F32R, kind="ExternalInput")
    wk_d = nc.dram_tensor("wk", [DIM, NKV_L * HD], F32R, kind="ExternalInput")
    wv_d = nc.dram_tensor("wv", [DIM, NKV_L * HD], F32R, kind="ExternalInput")
    wo_d = nc.dram_tensor("wo", [NH_L * HD, DIM], F32R, kind="ExternalInput")
    cosq_d = nc.dram_tensor("cosq", [P, S], F32, kind="ExternalInput")
    sinq_d = nc.dram_tensor("sinq", [P, S], F32, kind="ExternalInput")
    cosk_d = nc.dram_tensor("cosk", [P, S], F32, kind="ExternalInput")
    sink_d = nc.dram_tensor("sink", [P, S], F32, kind="ExternalInput")
    pmask_d = nc.dram_tensor("pmask", [P, P], BF16, kind="ExternalInput")
    out_d = nc.dram_tensor("out", [S, DIM], F32, kind="ExternalOutput")

    if debug:
        dbg_kt = nc.dram_tensor("dbg_kt", [P, NKV_L, S], F32, kind="ExternalOutput")
        dbg_v = nc.dram_tensor("dbg_v", [P, cfg.NKT, NKV_L * HD], BF16,
                               kind="ExternalOutput")
        dbg_qt = nc.dram_tensor("dbg_qt", [P, NH_L, S], F32, kind="ExternalOutput")
        dbg_at = nc.dram_tensor("dbg_at", [P, NH_L, S], F32, kind="ExternalOutput")

    xT_r = xT_d.ap().rearrange("(co ci) t -> ci co t", ci=P)
    wq_r = wq_d.ap().rearrange("(co ci) d -> ci co d", ci=P)
    wk_r = wk_d.ap().rearrange("(co ci) d -> ci co d", ci=P)
    wv_r = wv_d.ap().rearrange("(co ci) d -> ci co d", ci=P)
    wo_r = wo_d.ap().rearrange("(dc p) m -> p dc m", p=P)

    def r(ap):
        return ap if ap.dtype == F32R else ap.bitcast(F32R)

    def mm(out, lhsT, rhs, start, stop):
        nc.tensor.matmul(out, r(lhsT), r(rhs), start=start, stop=stop)

    def mmb(out, lhsT, rhs, start, stop):
        nc.tensor.matmul(out, lhsT, rhs, start=start, stop=stop)

    with tile.TileContext(nc) as tc, ExitStack() as top:
        const = top.enter_context(tc.tile_pool(name="const", bufs=1))
        pmask_sb = const.tile([P, P], BF16)
        nc.sync.dma_start(pmask_sb[:], pmask_d.ap())
        ones_row = const.tile([P, P], BF16)
        nc.vector.memset(ones_row[:], 1.0)

        kvp = top.enter_context(tc.tile_pool(name="kvp", bufs=1))
        KT_sb = kvp.tile([P, NKV_L, S], F32)
        V_sb = kvp.tile([P, cfg.NKT, NKV_L * HD], BF16)
        qtp = tc.alloc_tile_pool(name="qtp", bufs=1)
        qt_sb = qtp.tile([P, NH_L, S], F32)

        def rope_inplace(dst, cos_sl, sin_sl, tmp_pool):
            # dst [P, n] f32 in SBUF: dst = dst*cos + swap_halves(dst)*sin
            n = dst.shape[-1]
            tmp = tmp_pool.tile([P, TQ], F32, tag="ropetmp", name="ropetmp")
            t = tmp[:, :n]
            nc.sync.dma_start(t[0:64], dst[64:128])
            nc.sync.dma_start(t[64:128], dst[0:64])
            nc.vector.tensor_tensor(t.bitcast(F32R), t, sin_sl, MULT)
            nc.vector.tensor_tensor(dst.bitcast(F32R), dst, cos_sl, MULT)
            nc.vector.tensor_add(dst.bitcast(F32R), dst, t)

        # ---------------- Phase A: K^T and V projections (+ RoPE on K) -----
        # wk/wv chunks JIT-stream inside tn=0's c-loop; cos/sin K tables use
        # per-tn rolling tiles. V is written bf16 (it feeds the bf16 PV path).
        with ExitStack() as ctx:
            wkvp = ctx.enter_context(tc.tile_pool(name="wkvp", bufs=1))
            ktab = ctx.enter_context(tc.tile_pool(name="ktab", bufs=2))
            xap = ctx.enter_context(tc.tile_pool(name="xap", bufs=10))
            rtp = ctx.enter_context(tc.tile_pool(name="rtp", bufs=2))
            pka = ctx.enter_context(tc.tile_pool(name="pka", bufs=2, space="PSUM"))
            pva = ctx.enter_context(tc.tile_pool(name="pva", bufs=1, space="PSUM"))

            wk_sb = wkvp.tile([P, CCH, NKV_L * HD], F32R)
            wv_sb = wkvp.tile([P, CCH, NKV_L * HD], F32R)

            LOOK = 3
            for tn in range(NT):
                tsl = slice(tn * TQ, (tn + 1) * TQ)
                cosk_t = ktab.tile([P, TQ], F32, tag="ckt", name="ckt")
                sink_t = ktab.tile([P, TQ], F32, tag="skt", name="skt")
                nc.sync.dma_start(cosk_t[:], cosk_d.ap()[:, tsl])
                nc.sync.dma_start(sink_t[:], sink_d.ap()[:, tsl])
                psk = [pka.tile([P, TQ], F32, tag=f"psk{d}", name=f"psk{d}")
                       for d in range(NKV_L)]
                psv = [pva.tile([P, NKV_L * HD], F32, tag=f"psv{j}", name=f"psv{j}")
                       for j in range(RT)]
                for c in range(CCH):
                    if tn == 0:
                        if c == 0:
                            for cc in range(LOOK + 1):
                                nc.sync.dma_start(wk_sb[:, cc, :], wk_r[:, cc, :])
                                nc.sync.dma_start(wv_sb[:, cc, :], wv_r[:, cc, :])
                        elif c + LOOK < CCH:
                            cc = c + LOOK
                            nc.sync.dma_start(wk_sb[:, cc, :], wk_r[:, cc, :])
                            nc.sync.dma_start(wv_sb[:, cc, :], wv_r[:, cc, :])
                    xt = xap.tile([P, TQ], F32R, tag="xa", name="xa")
                    nc.sync.dma_start(xt[:], xT_r[:, c, tsl])
                    st, sp = c == 0, c == CCH - 1
                    for d in range(NKV_L):
                        mm(psk[d][:], wk_sb[:, c, d * HD:(d + 1) * HD], xt[:], st, sp)
                    for j in range(RT):
                        mm(psv[j][:], xt[:, j * P:(j + 1) * P], wv_sb[:, c, :], st, sp)
                for j in range(RT):
                    nc.scalar.copy(V_sb[:, tn * RT + j, :], psv[j][:])
                for d in range(NKV_L):
                    nc.scalar.copy(KT_sb[:, d, tsl].bitcast(F32R), psk[d][:])
                    rope_inplace(KT_sb[:, d, tsl], cosk_t[:], sink_t[:], rtp)

        # ---------------- Phase Q: Q^T projection (+ RoPE on Q) ------------
        # 2-level accumulation, NACC=2 groups of 16 c-chunks. wq streams as
        # JIT pieces of 4 chunks (5-deep pool); q cos/sin tables roll per tn.
        NACC = 2
        GC = CCH // NACC          # c-chunks per accumulation group (16)
        PCH = 4                   # chunks per wq piece
        NPC = GC // PCH           # pieces per group (4)
        with ExitStack() as ctx:
            wqp = ctx.enter_context(tc.tile_pool(name="wqp", bufs=NPC + 1))
            qtab = ctx.enter_context(tc.tile_pool(name="qtab", bufs=2))
            xqp = ctx.enter_context(tc.tile_pool(name="xqp", bufs=10))
            rtq = ctx.enter_context(tc.tile_pool(name="rtq", bufs=2))
            pqa = ctx.enter_context(tc.tile_pool(name="pqa", bufs=1, space="PSUM"))

            def fetch_piece(g, p):
                piece = wqp.tile([P, PCH, NH_L * HD], F32R, tag="wqs", name="wqs")
                for i in range(PCH):
                    c0 = g * GC + p * PCH + i
                    nc.sync.dma_start(piece[:, i, :], wq_r[:, c0, :])
                return piece

            pieces = [fetch_piece(0, p) for p in range(NPC)]
            nxt = []
            for g in range(NACC):
                for tn in range(NT):
                    tsl = slice(tn * TQ, (tn + 1) * TQ)
                    if g == NACC - 1:
                        # tables for this tn's rope (used at the tn's tail)
                        cq = qtab.tile([P, TQ], F32, tag="cqt", name="cqt")
                        sq = qtab.tile([P, TQ], F32, tag="sqt", name="sqt")
                        nc.sync.dma_start(cq[:], cosq_d.ap()[:, tsl])
                        nc.sync.dma_start(sq[:], sinq_d.ap()[:, tsl])
                    psq = [pqa.tile([P, TQ], F32, tag=f"psq{h}", name=f"psq{h}")
                           for h in range(NH_L)]
                    for ci in range(GC):
                        # prefetch next group's pieces during the last tn
                        if g + 1 < NACC and tn == NT - 1 and ci % PCH == 0:
                            nxt.append(fetch_piece(g + 1, ci // PCH))
                        piece = pieces[ci // PCH]
                        col = ci % PCH
                        xt = xqp.tile([P, TQ], F32R, tag="xq", name="xq")
                        nc.sync.dma_start(xt[:], xT_r[:, g * GC + ci, tsl])
                        st, sp = ci == 0, ci == GC - 1
                        for h in range(NH_L):
                            mm(psq[h][:], piece[:, col, h * HD:(h + 1) * HD],
                               xt[:], st, sp)
                    for h in range(NH_L):
                        if g == 0:
                            nc.scalar.copy(qt_sb[:, h, tsl].bitcast(F32R),
                                           psq[h][:])
                        else:
                            nc.vector.tensor_add(qt_sb[:, h, tsl].bitcast(F32R),
                                                 qt_sb[:, h, tsl], psq[h][:])
                        if g == NACC - 1:
                            rope_inplace(qt_sb[:, h, tsl], cq[:], sq[:], rtq)
                if nxt:
                    pieces, nxt = nxt, []

        if debug:
            nc.sync.dma_start(dbg_kt.ap(), KT_sb[:])
            nc.sync.dma_start(dbg_v.ap(), V_sb[:])
            nc.sync.dma_start(dbg_qt.ap(), qt_sb[:])

        # ---------------- Phase S: attention per head ----------------------
        # Software-pipelined: PV(kt) is emitted after scores(kt+L). Masks are
        # multiplicative on the bf16 exp tile. Denominator epilogue (ones_row
        # matmul -> broadcast sums in PSUM -> reciprocal -> normalize) is
        # deferred one block.
        atp = tc.alloc_tile_pool(name="atp", bufs=1, side="right")
        attnT_sb = atp.tile([P, NH_L, S], F32)
        with ExitStack() as ctx:
            ptp = ctx.enter_context(tc.tile_pool(name="ptp", bufs=6))
            acp = ctx.enter_context(tc.tile_pool(name="acp", bufs=3))
            bcp = ctx.enter_context(tc.tile_pool(name="bcp", bufs=2))
            psc = ctx.enter_context(tc.tile_pool(name="psc", bufs=4, space="PSUM"))
            pso = ctx.enter_context(tc.tile_pool(name="pso", bufs=3, space="PSUM"))
            pss = ctx.enter_context(tc.tile_pool(name="pss", bufs=1, space="PSUM"))

            LP = 3  # PV lookahead (score-matmuls emitted ahead of each PV)

            def epilogue(pend):
                h, qsl, ps_out, acc = pend
                ps_sum = pss.tile([P, TQ], F32, tag="pssum", name="pssum")
                mmb(ps_sum[:], ones_row[:], acc[:], True, True)
                bc_sb = bcp.tile([P, TQ], F32, tag="bcsb", name="bcsb")
                nc.vector.reciprocal_approx_fast(out=bc_sb[:], in_=ps_sum[:])
                nc.vector.tensor_tensor(attnT_sb[:, h, qsl].bitcast(F32R),
                                        ps_out[:], bc_sb[:], MULT)

            pending = None
            for qc in range(NT):
                for h in range(NH_L):
                    g = h // cfg.NREP
                    qsl = slice(qc * TQ, (qc + 1) * TQ)
                    ps_out = pso.tile([P, TQ], F32, tag="psout", name="psout")
                    acc = acp.tile([P, TQ], BF16, tag="acc", name="acc")
                    nkt = (qc + 1) * RT
                    pts = {}
                    for step in range(nkt + LP):
                        if step < nkt:
                            kt = step
                            ps_sc = psc.tile([P, TQ], F32, tag="pssc", name="pssc")
                            mm(ps_sc[:], KT_sb[:, g, kt * P:(kt + 1) * P],
                               qt_sb[:, h, qsl], True, True)
                            pt = ptp.tile([P, TQ], BF16, tag="pt", name="pt")
                            nc.scalar.activation(pt[:], ps_sc[:],
                                                 mybir.ActivationFunctionType.Exp)
                            if kt >= qc * RT:
                                qoff = (kt - qc * RT) * P
                                if qoff > 0:
                                    nc.vector.memset(pt[:, 0:qoff], 0.0)
                                nc.vector.tensor_tensor(pt[:, qoff:qoff + P],
                                                        pt[:, qoff:qoff + P],
                                                        pmask_sb[:], MULT)
                            if kt == 0:
                                nc.vector.tensor_copy(acc[:], pt[:])
                            else:
                                nc.vector.tensor_add(acc[:], acc[:], pt[:])
                            pts[kt] = pt
                        if step == min(2, nkt - 1) and pending is not None:
                            epilogue(pending)
                            pending = None
                        j = step - LP
                        if 0 <= j < nkt:
                            mmb(ps_out[:], V_sb[:, j, g * HD:(g + 1) * HD],
                                pts.pop(j)[:], j == 0, j == nkt - 1)
                    pending = (h, qsl, ps_out, acc)
            epilogue(pending)

        if debug:
            nc.sync.dma_start(dbg_at.ap(), attnT_sb[:])

        qtp.release()

        # ---------------- Phase W: output projection -----------------------
        # wo streams as JIT [P, 4, TQ] slabs (2 per mc column chunk).
        with ExitStack() as ctx:
            wop = ctx.enter_context(tc.tile_pool(name="wop", bufs=4, side="right"))
            owp = ctx.enter_context(tc.tile_pool(name="owp", bufs=3, side="right"))
            psw = ctx.enter_context(tc.tile_pool(name="psw", bufs=4, space="PSUM"))

            def fetch_wo(mc, dh):
                slab = wop.tile([P, 4, TQ], F32R, tag="wos", name="wos")
                msl = slice(mc * TQ, (mc + 1) * TQ)
                for i in range(4):
                    nc.sync.dma_start(slab[:, i, :], wo_r[:, dh * 4 + i, msl])
                return slab

            cur = [fetch_wo(0, 0), fetch_wo(0, 1)]
            for mc in range(DIM // TQ):
                msl = slice(mc * TQ, (mc + 1) * TQ)
                nxt = []
                for tb in range(S // P):
                    # prefetch next mc's two slabs early in this mc's tb loop
                    if mc + 1 < DIM // TQ and tb in (1, 3):
                        nxt.append(fetch_wo(mc + 1, len(nxt)))
                    ps_w = psw.tile([P, TQ], F32, tag="psw", name="psw")
                    for dc in range(NH_L):
                        mm(ps_w[:], attnT_sb[:, dc, tb * P:(tb + 1) * P],
                           cur[dc // 4][:, dc % 4, :], dc == 0, dc == NH_L - 1)
                    ot = owp.tile([P, TQ], F32, tag="ot", name="ot")
                    nc.scalar.copy(ot[:], ps_w[:])
                    nc.sync.dma_start(out_d.ap()[tb * P:(tb + 1) * P, msl], ot[:])
                if nxt:
                    cur = nxt

        atp.release()

    nc.compile()
    return nc


# ---------------------------------------------------------------------------
# Host side
# ---------------------------------------------------------------------------

_HALF_PERM = np.concatenate([np.arange(0, P, 2), np.arange(1, P, 2)])

LAST_EXEC_NS = None
LAST_RESULTS = None


def _host_prep(cfg: Cfg, x, wq, wk, wv, wo, freqs_cos, freqs_sin):
    """Build the 8 per-core input maps. Core c: batch c % 2, group c // 2."""
    import ml_dtypes
    BF = ml_dtypes.bfloat16

    B = x.shape[0]
    n_groups = wq.shape[1] // (cfg.NH_L * cfg.HD)
    hd = cfg.HD

    cosT = np.ascontiguousarray(freqs_cos.T.astype(np.float32))  # [HD/2, S]
    sinT = np.ascontiguousarray(freqs_sin.T.astype(np.float32))
    sc = np.float32(1.0 / math.sqrt(hd))
    cosq = np.concatenate([cosT, cosT], 0) * sc
    sinq = np.concatenate([-sinT, sinT], 0) * sc
    cosk = np.concatenate([cosT, cosT], 0)
    sink = np.concatenate([-sinT, sinT], 0)
    # multiplicative causal mask for the transposed diagonal block [k, q]:
    # allowed iff k_local <= q_local
    pmask = np.triu(np.ones((P, P), np.float32), 0).astype(BF)

    xT = [np.ascontiguousarray(x[b].T).astype(np.float32) for b in range(B)]

    def permute_cols(w, nheads):
        w = w.reshape(cfg.DIM, nheads, hd)[:, :, _HALF_PERM]
        return np.ascontiguousarray(w.reshape(cfg.DIM, nheads * hd), dtype=np.float32)

    in_maps = []
    qcols = cfg.NH_L * hd
    kcols = cfg.NKV_L * hd
    for c in range(B * n_groups):
        b, g = c % B, c // B
        in_maps.append(dict(
            xT=xT[b],
            wq=permute_cols(wq[:, g * qcols:(g + 1) * qcols], cfg.NH_L),
            wk=permute_cols(wk[:, g * kcols:(g + 1) * kcols], cfg.NKV_L),
            wv=np.ascontiguousarray(wv[:, g * kcols:(g + 1) * kcols], dtype=np.float32),
            wo=np.ascontiguousarray(wo[g * qcols:(g + 1) * qcols, :], dtype=np.float32),
            cosq=cosq, sinq=sinq, cosk=cosk, sink=sink, pmask=pmask,
        ))
    return in_maps


def kernel(x, wq, wk, wv, wo, freqs_cos, freqs_sin, mask, start_pos=0):
    global LAST_EXEC_NS, LAST_RESULTS
    x = np.asarray(x, np.float32)
    wq = np.asarray(wq, np.float32)
    wk = np.asarray(wk, np.float32)
    wv = np.asarray(wv, np.float32)
    wo = np.asarray(wo, np.float32)
    freqs_cos = np.asarray(freqs_cos, np.float32)
    freqs_sin = np.asarray(freqs_sin, np.float32)

    cfg = Cfg()
    B = x.shape[0]
    n_groups = 4
    in_maps = _host_prep(cfg, x, wq, wk, wv, wo, freqs_cos, freqs_sin)

    from concourse.bass_utils import run_bass_kernel_spmd

    debug = bool(int(os.environ.get("KERNEL_DEBUG", "0")))
    nc = build_program(cfg, debug=debug)
    trace = bool(int(os.environ.get("KERNEL_TRACE", "0")))
    res = run_bass_kernel_spmd(nc, in_maps, core_ids=list(range(len(in_maps))),
                               trace=trace)
    LAST_EXEC_NS = res.exec_time_ns
    LAST_RESULTS = res

    out = np.zeros((B, cfg.S, cfg.DIM), np.float32)
    for c in range(B * n_groups):
        b = c % B
        out[b] += res.results[c]["out"]
    return out
